# revision 1
# baseline (speedup 1.0000x reference)
"""Multi-head self-attention (B=4, S=2048, D=1024, H=16) on 8 NeuronCores.

Sharding: data-parallel over batch (4 groups) x tensor-parallel over heads
(2 groups of 8 heads).  Core c handles batch b=c//2, head-group g=c%2.
Each core computes its 8 heads' attention plus a partial out-projection;
the host sums the two partials per batch, transposes, adds out_b.

Per-core kernel:
  - all matmul-fed tensors are float32r (fp32 rounded to 12-bit mantissa)
    so the PE runs single-pass full rate (plain fp32 matmul is 4x slower);
    inputs are pre-rounded on the host; PSUM accumulation stays fp32
  - q^T,k^T in [feature, token] layout (lhsT = W tile, rhs = x^T);
    v in [token, feature] layout (lhsT = x^T tile, rhs = W^T), augmented
    with a per-head ones column (zero wv columns + 1.0 bias entries)
  - scores^T per head via row-packed pair matmuls (two K=64 heads occupy
    row groups 0-1 / 2-3 concurrently; fp32r forbids column tiling)
  - softmax: exp on ScalarE with the 1/sqrt(hd) scale folded in (no max
    subtraction: scores ~ N(0,1), fp32-safe); denominators ride the AV
    matmul as row 64 of the M=65 stationary (the ones column);
    normalization = DVE copies + denominator broadcast via a DRAM bounce
    (partition-stride-0 DRAM reads) + reciprocal_approx_fast + multiply
  - pipeline: v projection upfront (PE warmup), then per pair q/k
    projection interleaved between the previous pair's ACT-bound
    attention chunks; o^T staged through DRAM; final out-projection
    contracts the 512 local head dims into a tiled partial output
Weights/outputs use host-prepacked tiled layouts so every DMA is
contiguous; walrus requires Bacc.compile() for the 1-wait-per-
instruction sync legalization.
"""

import numpy as np

_B, _S, _D, _H = 4, 2048, 1024, 16
_FH = 512  # local feature dims per core (8 heads x 64)
_ND = _D // 128
_NPAIR = _FH // 128
_NCORES = 8

_CACHE = {}


def _build(S):
    import concourse.bass as bass
    import concourse.bacc as bacc
    import concourse.tile as tile
    import concourse.mybir as mybir
    from contextlib import ExitStack

    f32 = mybir.dt.float32
    f32r = mybir.dt.float32r
    Exp = mybir.ActivationFunctionType.Exp
    D, FH = _D, _FH
    ND = D // 128            # contraction tiles for the projections
    NPAIR = FH // 128        # head pairs
    NKT = S // 128           # key tiles
    CH = min(1024, S)        # tq chunk (psum tile free size)
    NCH = S // CH
    HW = min(512, CH)        # matmul moving free dim
    NHALF = CH // HW
    TS = min(512, S)         # projection t-slice
    NTS = S // TS
    NH = FH // 64            # local heads
    FHA = NH * 65            # v width incl. per-head ones column

    nc = bacc.Bacc("TRN2", target_bir_lowering=False, debug=False)

    xT_d = nc.dram_tensor("xT", [D, S], f32r, kind="ExternalInput")
    wq_d = nc.dram_tensor("wq", [NPAIR, 128, ND, 128], f32r, kind="ExternalInput")
    wk_d = nc.dram_tensor("wk", [NPAIR, 128, ND, 128], f32r, kind="ExternalInput")
    wv_d = nc.dram_tensor("wv", [128, ND, FHA], f32r, kind="ExternalInput")
    wo_d = nc.dram_tensor("wo", [128, NPAIR, D], f32r, kind="ExternalInput")
    bq_d = nc.dram_tensor("bq", [128, NPAIR], f32, kind="ExternalInput")
    bk_d = nc.dram_tensor("bk", [128, NPAIR], f32, kind="ExternalInput")
    bv_d = nc.dram_tensor("bv", [1, FHA], f32r, kind="ExternalInput")
    onr_d = nc.dram_tensor("onesr", [1, 128], f32r, kind="ExternalInput")
    outp_d = nc.dram_tensor("outp", [ND, NTS, 128, TS], f32, kind="ExternalOutput")
    otn_d = nc.dram_tensor("otn_scr", [128, NPAIR, S], f32r)
    v_d = nc.dram_tensor("v_scr", [NPAIR, 128, NKT, 130], f32r)

    with tile.TileContext(nc) as tc, ExitStack() as top:
        consts = top.enter_context(tc.tile_pool(name="consts", bufs=1))
        ps = top.enter_context(tc.tile_pool(name="ps", bufs=4, space="PSUM"))

        ones_row = consts.tile([1, 128], f32r)
        nc.sync.dma_start(out=ones_row, in_=onr_d[:])
        bqk_sb = consts.tile([128, 2 * NPAIR], f32)
        nc.sync.dma_start(out=bqk_sb[:, 0:NPAIR], in_=bq_d[:])
        nc.sync.dma_start(out=bqk_sb[:, NPAIR:2 * NPAIR], in_=bk_d[:])
        bv_sb = consts.tile([1, FHA], f32r)
        nc.sync.dma_start(out=bv_sb, in_=bv_d[:])
        # dummy exp so the ACT table set loads during the ramp, not at the
        # first real softmax exp inside the attention window
        warm = consts.tile([1, 8], f32)
        nc.vector.memset(warm, 0.0)
        nc.scalar.activation(out=warm, in_=warm, func=Exp)

        qkT_pool = top.enter_context(tc.tile_pool(name="qk", bufs=1))
        qkT = qkT_pool.tile([128, NPAIR, 2, S], f32r)      # [f%128, pair, q/k, t]
        vstream0 = top.enter_context(tc.tile_pool(name="vstream", bufs=2))
        with tc.tile_pool(name="xtp", bufs=1) as xtp:
            xT_sb = xtp.tile([128, ND, S], f32r)
            XC = min(512, S)
            for c in range(S // XC):      # t-major so early tiles land first
                for d in range(ND):
                    nc.sync.dma_start(
                        out=xT_sb[:, d, c * XC:(c + 1) * XC],
                        in_=xT_d[d * 128:(d + 1) * 128, c * XC:(c + 1) * XC],
                    )

            # ----- v projection (PE-heavy warmup; v staged to DRAM,
            # except pair 0 which fills its SBUF tile directly) -----
            v_p0 = vstream0.tile([128, NKT, 130], f32r, tag="vp")
            with ExitStack() as phv:
                wv_pool = phv.enter_context(tc.tile_pool(name="wvp", bufs=1))
                vs_pool = phv.enter_context(tc.tile_pool(name="vsg", bufs=3))
                wv_sb = wv_pool.tile([128, ND, FHA], f32r)
                for d in range(ND):
                    nc.sync.dma_start(out=wv_sb[:, d, :], in_=wv_d[:, d, :])
                vsplits = [(0, min(512, FHA))]
                if FHA > 512:
                    vsplits.append((512, FHA - 512))
                for t in range(NKT):
                    vps = ps.tile([128, FHA], f32, tag="ps")
                    for c0, cw in vsplits:
                        for d in range(ND):
                            nc.tensor.matmul(
                                vps[:, c0:c0 + cw],
                                lhsT=xT_sb[:, d, t * 128:(t + 1) * 128],
                                rhs=wv_sb[:, d, c0:c0 + cw],
                                start=(d == 0),
                                stop=False,
                            )
                        nc.tensor.matmul(
                            vps[:, c0:c0 + cw], lhsT=ones_row,
                            rhs=bv_sb[:, c0:c0 + cw], start=False, stop=True,
                        )
                    nc.vector.tensor_copy(
                        out=v_p0[:, t, :], in_=vps[:, 0:130])
                    v_stage = vs_pool.tile([128, FHA - 130], f32r, tag="vs")
                    nc.scalar.copy(out=v_stage, in_=vps[:, 130:FHA])
                    for p in range(1, NPAIR):
                        nc.sync.dma_start(
                            out=v_d[p, :, t, :],
                            in_=v_stage[:, (p - 1) * 130:p * 130],
                        )

            # ----- per pair: q/k projection then attention -----
            with ExitStack() as ph2ctx:
                wstream = ph2ctx.enter_context(tc.tile_pool(name="wstream", bufs=2))
                vstream = vstream0
                pt_pool = ph2ctx.enter_context(tc.tile_pool(name="pt", bufs=3))
                ab_pool = ph2ctx.enter_context(tc.tile_pool(name="ab", bufs=2))
                r_pool = ph2ctx.enter_context(tc.tile_pool(name="r", bufs=1))
                otn_pool = ph2ctx.enter_context(tc.tile_pool(name="otn", bufs=2))
                dr_pool = ph2ctx.enter_context(
                    tc.tile_pool(name="dr", bufs=2, space="DRAM"))

                def qkproj(p, jlist, w_tiles):
                    wq_sb, wk_sb = w_tiles[0], w_tiles[1]
                    for j in jlist:
                        for which, w_sb in ((0, wq_sb), (1, wk_sb)):
                            pps = ps.tile([128, TS], f32, tag="ps")
                            for d in range(ND):
                                nc.tensor.matmul(
                                    pps,
                                    lhsT=w_sb[:, d, :],
                                    rhs=xT_sb[:, d, j * TS:(j + 1) * TS],
                                    start=(d == 0),
                                    stop=(d == ND - 1),
                                )
                            nc.vector.tensor_scalar_add(
                                out=qkT[:, p, which, j * TS:(j + 1) * TS],
                                in0=pps,
                                scalar1=bqk_sb[:, which * NPAIR + p:
                                               which * NPAIR + p + 1],
                            )

                def load_pair(p):
                    wq_sb = wstream.tile([128, ND, 128], f32r, tag="wq")
                    nc.sync.dma_start(out=wq_sb, in_=wq_d[p])
                    wk_sb = wstream.tile([128, ND, 128], f32r, tag="wk")
                    nc.sync.dma_start(out=wk_sb, in_=wk_d[p])
                    if p == 0:
                        v_p = v_p0
                    else:
                        v_p = vstream.tile([128, NKT, 130], f32r, tag="vp")
                        nc.sync.dma_start(out=v_p, in_=v_d[p])
                    return (wq_sb, wk_sb), v_p

                # interleave: pair p+1's q/k projection is emitted between
                # pair p's attention chunks so the PE fills ACT-bound gaps
                JPC = max(1, NTS // NCH)
                TPC = max(1, NKT // NCH)
                w_cur, v_cur = load_pair(0)
                qkproj(0, range(NTS), w_cur)
                w_nxt = v_nxt = None
                for p in range(NPAIR):
                    v_p = v_cur
                    if p + 1 < NPAIR:
                        w_nxt, v_nxt = load_pair(p + 1)

                    # attention for this pair
                    for ch in range(NCH):
                        t0 = ch * CH
                        # per-head o accumulators: rows 0:64 = o, row 64 =
                        # softmax denominator (ones column of augmented v)
                        oA = ps.tile([128, CH], f32, tag="ps")
                        oB = ps.tile([128, CH], f32, tag="ps")
                        for i in range(NKT):
                            sA = ps.tile([128, CH], f32, tag="ps")
                            sB = ps.tile([128, CH], f32, tag="ps")
                            kslc = slice(i * 128, (i + 1) * 128)
                            for h in range(NHALF):
                                q0 = t0 + h * HW
                                nc.tensor.matmul(
                                    sA[:, h * HW:(h + 1) * HW],
                                    lhsT=qkT[0:64, p, 1, kslc],
                                    rhs=qkT[0:64, p, 0, q0:q0 + HW],
                                    start=True, stop=True,
                                    tile_position=(0, 0),
                                )
                                nc.tensor.matmul(
                                    sB[:, h * HW:(h + 1) * HW],
                                    lhsT=qkT[64:128, p, 1, kslc],
                                    rhs=qkT[64:128, p, 0, q0:q0 + HW],
                                    start=True, stop=True,
                                    tile_position=(64, 0),
                                )
                            ptA = pt_pool.tile([128, CH], f32r, tag="pt")
                            nc.scalar.activation(
                                out=ptA, in_=sA, func=Exp, scale=0.125
                            )
                            ptB = pt_pool.tile([128, CH], f32r, tag="pt")
                            nc.scalar.activation(
                                out=ptB, in_=sB, func=Exp, scale=0.125
                            )
                            first, last = (i == 0), (i == NKT - 1)
                            for h in range(NHALF):
                                hs = slice(h * HW, (h + 1) * HW)
                                nc.tensor.matmul(
                                    oA[0:65, hs],
                                    lhsT=v_p[:, i, 0:65],
                                    rhs=ptA[:, hs],
                                    start=first, stop=last,
                                )
                                nc.tensor.matmul(
                                    oB[0:65, hs],
                                    lhsT=v_p[:, i, 65:130],
                                    rhs=ptB[:, hs],
                                    start=first, stop=last,
                                )
                        # normalize: copy to SBUF, broadcast denominators
                        # via a DRAM bounce, reciprocal, multiply
                        aS = ab_pool.tile([128, CH], f32, tag="ab")
                        nc.vector.tensor_copy(out=aS[0:65, :], in_=oA[0:65, :])
                        bS = ab_pool.tile([128, CH], f32, tag="ab")
                        nc.vector.tensor_copy(out=bS[0:65, :], in_=oB[0:65, :])
                        dscr = dr_pool.tile([2, CH], f32, tag="d")
                        nc.sync.dma_start(out=dscr[0:1, :], in_=aS[64:65, :])
                        nc.sync.dma_start(out=dscr[1:2, :], in_=bS[64:65, :])
                        # assemble both heads' o bodies in aS
                        nc.sync.dma_start(out=aS[64:128, :], in_=bS[0:64, :])
                        rS = r_pool.tile([128, CH], f32, tag="rs")
                        nc.sync.dma_start(
                            out=rS[0:64, :],
                            in_=dscr[0:1, :].to_broadcast([64, CH]))
                        nc.sync.dma_start(
                            out=rS[64:128, :],
                            in_=dscr[1:2, :].to_broadcast([64, CH]))
                        rR = r_pool.tile([128, CH], f32, tag="rr")
                        nc.vector.reciprocal_approx_fast(out=rR, in_=rS)
                        otn_t = otn_pool.tile([128, CH], f32r, tag="otn")
                        nc.vector.tensor_mul(out=otn_t, in0=aS, in1=rR)
                        nc.sync.dma_start(
                            out=otn_d[:, p, t0:t0 + CH], in_=otn_t
                        )
                        if p + 1 < NPAIR:
                            jl = range(ch * JPC, min((ch + 1) * JPC, NTS))
                            qkproj(p + 1, jl, w_nxt)
                    if p + 1 < NPAIR and NCH * JPC < NTS:
                        qkproj(p + 1, range(NCH * JPC, NTS), w_nxt)
                    w_cur, v_cur = w_nxt, v_nxt

        # ----- out projection -----
        with ExitStack() as ph3ctx:
            ph3 = ph3ctx.enter_context(tc.tile_pool(name="ph3", bufs=1))
            st_pool = ph3ctx.enter_context(tc.tile_pool(name="st", bufs=3))
            wo_sb = ph3.tile([128, NPAIR, D], f32r)
            nc.sync.dma_start(out=wo_sb, in_=wo_d[:])
            otn_rd = ph3.tile([128, NPAIR, S], f32r)
            for p in range(NPAIR):
                for ch in range(NCH):
                    nc.sync.dma_start(
                        out=otn_rd[:, p, ch * CH:(ch + 1) * CH],
                        in_=otn_d[:, p, ch * CH:(ch + 1) * CH])
            for et in range(ND):
                for j in range(NTS):
                    ops = ps.tile([128, TS], f32, tag="ps")
                    for p in range(NPAIR):
                        nc.tensor.matmul(
                            ops,
                            lhsT=wo_sb[:, p, et * 128:(et + 1) * 128],
                            rhs=otn_rd[:, p, j * TS:(j + 1) * TS],
                            start=(p == 0),
                            stop=(p == NPAIR - 1),
                        )
                    st = st_pool.tile([128, TS], f32, tag="st")
                    nc.scalar.copy(out=st, in_=ops)
                    nc.sync.dma_start(out=outp_d[et, j], in_=st)

    nc.compile()
    return nc


def _get_nc(S=_S):
    if S not in _CACHE:
        _CACHE[S] = _build(S)
    return _CACHE[S]


def _c32(a):
    return np.ascontiguousarray(a, dtype=np.float32)


def _round_f32r(a):
    """Round fp32 -> nearest fp32r (12-bit mantissa) so PE fp32r matmuls
    see properly rounded operands. Falls back to raw bits if the
    neuron_dtypes cast helper is unavailable."""
    a = _c32(a)
    try:
        from neuron_dtypes._impl.fp32r import cast_fp32_to_fp32r
        flat = a.reshape(-1).view(np.uint32)
        out = np.asarray(cast_fp32_to_fp32r(flat.size, flat), dtype=np.uint32)
        return np.ascontiguousarray(out.view(np.float32).reshape(a.shape))
    except Exception:
        return a


def make_in_map(xT, wqT, wkT, wvT, woT, bq, bk, bv):
    """Pack one core's inputs into the kernel's tiled DRAM layouts.

    xT: [D, S] (x transposed); wqT/wkT/wvT: [D, FH] (W sections
    transposed); woT: [FH, D] (out_w columns transposed); biases: [FH].
    """
    D, FH, ND, NPAIR = _D, _FH, _ND, _NPAIR
    NH = FH // 64
    FHA = NH * 65
    # augment v with a per-head ones column: wv gets zero columns, bv gets
    # 1.0 entries -> the bias matmul produces the ones column, whose AV
    # accumulation yields the softmax denominators for free
    wva = np.zeros((D, FHA), dtype=np.float32)
    bva = np.zeros((1, FHA), dtype=np.float32)
    for h in range(NH):
        wva[:, h * 65:h * 65 + 64] = np.asarray(wvT)[:, h * 64:(h + 1) * 64]
        bva[0, h * 65:h * 65 + 64] = np.asarray(bv)[h * 64:(h + 1) * 64]
        bva[0, h * 65 + 64] = 1.0
    return {
        "xT": _round_f32r(xT),
        "wq": _round_f32r(np.asarray(wqT).reshape(ND, 128, NPAIR, 128).transpose(2, 1, 0, 3)),
        "wk": _round_f32r(np.asarray(wkT).reshape(ND, 128, NPAIR, 128).transpose(2, 1, 0, 3)),
        "wv": _round_f32r(wva.reshape(ND, 128, FHA).transpose(1, 0, 2)),
        "wo": _round_f32r(np.asarray(woT).reshape(NPAIR, 128, D).transpose(1, 0, 2)),
        "bq": _c32(np.asarray(bq).reshape(_NPAIR, 128).T),
        "bk": _c32(np.asarray(bk).reshape(_NPAIR, 128).T),
        "bv": _round_f32r(bva),
        "onesr": np.ones((1, 128), dtype=np.float32),
    }


def unpack_out(outp_tiled, S=_S):
    """[ND, NTS, 128, TS] tiled partial -> [D, S]."""
    ND = _ND
    NTS = S // min(512, S)
    return outp_tiled.transpose(0, 2, 1, 3).reshape(_D, S)


def _shard_inputs(x, in_proj_weight, in_proj_bias, out_w):
    w = np.asarray(in_proj_weight)
    b = np.asarray(in_proj_bias)
    ow = np.asarray(out_w)
    in_maps = []
    for c in range(_NCORES):
        bi, g = divmod(c, 2)
        sl = slice(g * _FH, (g + 1) * _FH)
        in_maps.append(make_in_map(
            xT=np.asarray(x[bi]).T,
            wqT=w[0 * _D:1 * _D][sl].T,
            wkT=w[1 * _D:2 * _D][sl].T,
            wvT=w[2 * _D:3 * _D][sl].T,
            woT=ow[:, sl].T,
            bq=b[0 * _D:1 * _D][sl],
            bk=b[1 * _D:2 * _D][sl],
            bv=b[2 * _D:3 * _D][sl],
        ))
    return in_maps


LAST_RESULTS = None


def kernel(x, in_proj_weight, in_proj_bias, out_w, out_b):
    global LAST_RESULTS
    from concourse.bass_utils import run_bass_kernel_spmd
    import os

    nc = _get_nc()
    in_maps = _shard_inputs(x, in_proj_weight, in_proj_bias, out_w)
    trace = os.environ.get("BASS_TRACE", "0") not in ("", "0")
    res = run_bass_kernel_spmd(
        nc, in_maps, core_ids=list(range(_NCORES)), trace=trace
    )
    LAST_RESULTS = res
    out_b = np.asarray(out_b, dtype=np.float32)
    out = np.empty((_B, _S, _D), dtype=np.float32)
    for b in range(_B):
        part = (unpack_out(res.results[2 * b]["outp"])
                + unpack_out(res.results[2 * b + 1]["outp"]))
        out[b] = part.T + out_b
    return out



# revision 8
# speedup vs baseline: 1.5293x; 1.5293x over previous
"""Multi-head self-attention (B=4, S=2048, D=1024, H=16) on 8 NeuronCores.

Sharding: data-parallel over batch (4 groups) x tensor-parallel over heads
(2 groups of 8 heads).  Core c handles batch b=c//2, head-group g=c%2.
Each core computes its 8 heads' attention plus a partial out-projection;
the host sums the two partials per batch, transposes, adds out_b.

v2 design (vs v1): everything in bf16 (plenty of margin vs the 2e-2 gate),
no DRAM staging, and a "flipped" AV matmul that halves the PE rows:

  - scores^T per head-pair via row-packed K=64 matmuls, psum tile
    [128 keys, 2*CH] holds both heads; ONE exp per ktile ([128, 1024]
    activation, scale=1/8 folded) -> pt [keys, q] bf16 in SBUF
  - AV flipped: stationary = pt q-block [128 keys, 128 q], moving = v_aug
    [128 keys, 65] (64 v dims + ones column) -> psum [128 q, 65] per
    (qblock, head), accumulated over the 16 ktiles.  65 moving rows per
    ktile instead of 128 q rows: ~2x fewer PE cycles than v1's AV.  The
    ones column (installed by a broadcast bias add on the v projection)
    lands the softmax denominator in psum column 64, per PARTITION
    (= per q), so normalization is a native tensor_scalar multiply.
  - PSUM accumulation groups must run ONE AT A TIME per psum bank
    (a start=True while another group is open in the same bank resets the
    bank - verified on hw).  So AV for chunk (c) runs as a post-pass
    (one (qblock, head) group after another) in the NEXT chunk's window,
    with pt double-buffered.
  - o comes out [q, feat]; out-projection needs o^T, done with cheap PE
    transposes (128 rows each) through PSUM.
  - schedule: pair-outer; pair p+1's q/k/v projections and chunk c-1's
    out-projection interleave into pair p's ACT-bound attention windows.
    ACT does only the 256 exps (~266us); PE ~280us; DVE does all
    PSUM->SBUF moves + bias/normalize (~110us).  gpsimd cannot read PSUM
    (walrus codegen fails) so DVE carries the copies.
"""

import numpy as np

_B, _S, _D, _H = 4, 2048, 1024, 16
_FH = 512        # local feature dims per core (8 heads x 64)
_ND = _D // 128  # contraction tiles
_NPAIR = 4       # head pairs (2 heads of 64 -> 128 features)
_NKT = _S // 128 # key tiles
_CH = 512        # q chunk
_NCH = _S // _CH
_NQB = _CH // 128
_NH = 8          # local heads
_FHA = _NH * 65  # v width incl. per-head ones column
_NCORES = 8

_CACHE = {}
_DEBUG = False  # adds qkT/v/o debug outputs to the kernel


def _build():
    import concourse.bass as bass
    import concourse.bacc as bacc
    import concourse.tile as tile
    import concourse.mybir as mybir
    from contextlib import ExitStack

    f32 = mybir.dt.float32
    bf16 = mybir.dt.bfloat16
    Exp = mybir.ActivationFunctionType.Exp
    D, S, FH, ND, NPAIR, NKT, CH, NCH, NQB, FHA = (
        _D, _S, _FH, _ND, _NPAIR, _NKT, _CH, _NCH, _NQB, _FHA)

    nc = bacc.Bacc("TRN2", target_bir_lowering=False, debug=False)

    xP_d = nc.dram_tensor("xP", [NCH, 128, ND, CH], bf16, kind="ExternalInput")
    wq_d = nc.dram_tensor("wq", [NPAIR, 128, ND, 128], bf16, kind="ExternalInput")
    wk_d = nc.dram_tensor("wk", [NPAIR, 128, ND, 128], bf16, kind="ExternalInput")
    wv_d = nc.dram_tensor("wv", [128, ND, FHA], bf16, kind="ExternalInput")
    wo_d = nc.dram_tensor("wo", [128, NPAIR, D], bf16, kind="ExternalInput")
    bq_d = nc.dram_tensor("bq", [128, NPAIR], f32, kind="ExternalInput")
    bk_d = nc.dram_tensor("bk", [128, NPAIR], f32, kind="ExternalInput")
    bv_d = nc.dram_tensor("bv", [1, FHA], bf16, kind="ExternalInput")
    idn_d = nc.dram_tensor("idn", [128, 128], bf16, kind="ExternalInput")
    outp_d = nc.dram_tensor("outp", [NCH, ND, 128, CH], bf16, kind="ExternalOutput")
    if _DEBUG:
        dbg_qkT = nc.dram_tensor("dbg_qkT", [128, NPAIR, 2, S], bf16, kind="ExternalOutput")
        dbg_v = nc.dram_tensor("dbg_v", [128, NKT, FHA], bf16, kind="ExternalOutput")
        dbg_o = nc.dram_tensor("dbg_o", [128, S // 128, FH], bf16, kind="ExternalOutput")

    with tile.TileContext(nc) as tc, ExitStack() as top:
        consts = top.enter_context(tc.tile_pool(name="consts", bufs=1))
        ps = top.enter_context(tc.tile_pool(name="ps", bufs=2, space="PSUM"))
        big = top.enter_context(tc.tile_pool(name="big", bufs=1))
        ptp = top.enter_context(tc.tile_pool(name="ptp", bufs=2))
        wst = top.enter_context(tc.tile_pool(name="wst", bufs=2))
        otp = top.enter_context(tc.tile_pool(name="otp", bufs=2))
        stp = top.enter_context(tc.tile_pool(name="stp", bufs=3))
        rcpp = top.enter_context(tc.tile_pool(name="rcpp", bufs=2))

        idn_sb = consts.tile([128, 128], bf16)
        nc.sync.dma_start(out=idn_sb, in_=idn_d[:])
        bqk_sb = consts.tile([128, 2 * NPAIR], f32)
        nc.sync.dma_start(out=bqk_sb[:, 0:NPAIR], in_=bq_d[:])
        nc.sync.dma_start(out=bqk_sb[:, NPAIR:2 * NPAIR], in_=bk_d[:])
        # v bias broadcast to all partitions (includes the 1.0 ones-column
        # entries that seed the softmax-denominator trick)
        bvb_sb = consts.tile([128, FHA], bf16)
        nc.sync.dma_start(out=bvb_sb, in_=bv_d[:].to_broadcast([128, FHA]))
        # dummy exp so the ACT table set loads during the ramp
        warm = consts.tile([1, 8], f32)
        nc.vector.memset(warm, 0.0)
        nc.scalar.activation(out=warm, in_=warm, func=Exp)

        xT_sb = big.tile([128, ND, S], bf16)
        qkT = big.tile([128, NPAIR, 2, S], bf16)  # [feat%128, pair, q/k, t]
        v_sb = big.tile([128, NKT, FHA], bf16)    # [token%128, ktile, head*65]
        o_sb = big.tile([128, S // 128, FH], bf16)  # [q%128, qblock, feat]
        wv_sb = big.tile([128, ND, FHA], bf16)
        wo_sb = big.tile([128, NPAIR, D], bf16)

        def load_w(p):
            wq_sb = wst.tile([128, ND, 128], bf16, tag="wq")
            nc.sync.dma_start(out=wq_sb, in_=wq_d[p])
            wk_sb = wst.tile([128, ND, 128], bf16, tag="wk")
            nc.sync.dma_start(out=wk_sb, in_=wk_d[p])
            return wq_sb, wk_sb

        w_cur = load_w(0)
        # x loads t-major (one DMA per 512-token slice) so the first qkproj
        # slice can start early
        for ts in range(NCH):
            nc.sync.dma_start(
                out=xT_sb[:, :, ts * CH:(ts + 1) * CH], in_=xP_d[ts])
            if ts == 0:
                nc.sync.dma_start(out=wv_sb, in_=wv_d[:])
        nc.sync.dma_start(out=wo_sb, in_=wo_d[:])

        def qkproj_slice(p, j, which, w_sb):
            pps = ps.tile([128, CH], f32, tag="mix")
            for d in range(ND):
                nc.tensor.matmul(
                    pps,
                    lhsT=w_sb[:, d, :],
                    rhs=xT_sb[:, d, j * CH:(j + 1) * CH],
                    start=(d == 0),
                    stop=(d == ND - 1),
                )
            nc.vector.tensor_scalar_add(
                out=qkT[:, p, which, j * CH:(j + 1) * CH],
                in0=pps,
                scalar1=bqk_sb[:, which * NPAIR + p:which * NPAIR + p + 1],
            )

        def vproj_t(p, t):
            vps = ps.tile([128, 130], f32, tag="mix")
            for d in range(ND):
                nc.tensor.matmul(
                    vps,
                    lhsT=xT_sb[:, d, t * 128:(t + 1) * 128],
                    rhs=wv_sb[:, d, p * 130:(p + 1) * 130],
                    start=(d == 0),
                    stop=(d == ND - 1),
                )
            nc.vector.tensor_add(
                out=v_sb[:, t, p * 130:(p + 1) * 130],
                in0=vps,
                in1=bvb_sb[:, p * 130:(p + 1) * 130],
            )

        def scores_block(p, c, pt_cur):
            for i in range(NKT):
                sAB = ps.tile([128, 2 * CH], f32, tag="sab")
                nc.tensor.matmul(
                    sAB[:, 0:CH],
                    lhsT=qkT[0:64, p, 1, i * 128:(i + 1) * 128],
                    rhs=qkT[0:64, p, 0, c * CH:(c + 1) * CH],
                    start=True, stop=True,
                    tile_position=(0, 0),
                )
                nc.tensor.matmul(
                    sAB[:, CH:2 * CH],
                    lhsT=qkT[64:128, p, 1, i * 128:(i + 1) * 128],
                    rhs=qkT[64:128, p, 0, c * CH:(c + 1) * CH],
                    start=True, stop=True,
                    tile_position=(64, 0),
                )
                nc.scalar.activation(
                    out=pt_cur[:, i, :], in_=sAB, func=Exp, scale=0.125)

        def av_norm_block(p, c, pt_prev):
            """AV + normalize for chunk (p, c), whose pt is complete.

            One accumulation group at a time per psum bank: the av01 bank
            hosts groups (qb0,h0), (qb0,h1), (qb1,h0), (qb1,h1) in
            sequence; av23 likewise (hardware resets an open group's bank
            on a concurrent start).
            """
            av_t = [ps.tile([128, 2, 130], f32, tag="av", name=f"av{half}")
                    for half in range(2)]
            rcp_t = rcpp.tile([128, 2, 4], f32, tag="rcp")
            for half in range(2):
                for qbl in range(2):
                    qb = half * 2 + qbl
                    for h in range(2):
                        for i in range(NKT):
                            nc.tensor.matmul(
                                av_t[half][:, qbl, h * 65:(h + 1) * 65],
                                lhsT=pt_prev[:, i, h * CH + qb * 128:
                                             h * CH + (qb + 1) * 128],
                                rhs=v_sb[:, i, p * 130 + h * 65:
                                         p * 130 + (h + 1) * 65],
                                start=(i == 0), stop=(i == NKT - 1),
                            )
            for half in range(2):
                for h in range(2):
                    nc.vector.reciprocal_approx_fast(
                        out=rcp_t[:, half, 2 * h:2 * h + 1],
                        in_=av_t[half][:, 0:1, 64 + 65 * h:65 + 65 * h],
                    )
                    nc.vector.reciprocal_approx_fast(
                        out=rcp_t[:, half, 2 * h + 1:2 * h + 2],
                        in_=av_t[half][:, 1:2, 64 + 65 * h:65 + 65 * h],
                    )
            for qb in range(NQB):
                avt = av_t[qb // 2]
                for h in range(2):
                    nc.vector.tensor_scalar_mul(
                        out=o_sb[:, c * NQB + qb,
                                 p * 128 + h * 64:p * 128 + (h + 1) * 64],
                        in0=avt[:, qb % 2, h * 65:h * 65 + 64],
                        scalar1=rcp_t[:, qb // 2,
                                      2 * h + qb % 2:2 * h + qb % 2 + 1],
                    )

        def emit_out_units(c):
            """Transposes + out-projection for chunk c, as thunks.

            tps units for fb<3 depend only on pairs 0-2 (whose chunk-c
            norms ran long ago); fb==3 waits on pair 3's norm.
            """
            oT = otp.tile([128, NPAIR, CH], bf16, tag="ot")

            def tps_unit(fb):
                def go():
                    tps = ps.tile([128, NQB, 128], bf16, tag="mix")
                    for qb in range(NQB):
                        nc.tensor.transpose(
                            out=tps[:, qb, :],
                            in_=o_sb[:, c * NQB + qb, fb * 128:(fb + 1) * 128],
                            identity=idn_sb,
                        )
                    nc.vector.tensor_copy(out=oT[:, fb, :], in_=tps)
                return go

            def ops_unit(et, on_act):
                def go():
                    ops = ps.tile([128, CH], f32, tag="mix")
                    for pb in range(NPAIR):
                        nc.tensor.matmul(
                            ops,
                            lhsT=wo_sb[:, pb, et * 128:(et + 1) * 128],
                            rhs=oT[:, pb, :],
                            start=(pb == 0),
                            stop=(pb == NPAIR - 1),
                        )
                    st = stp.tile([128, CH], bf16, tag="st")
                    if on_act:
                        nc.scalar.copy(out=st, in_=ops)
                    else:
                        nc.vector.tensor_copy(out=st, in_=ops)
                    nc.sync.dma_start(out=outp_d[c, et], in_=st)
                return go

            units = [tps_unit(fb) for fb in range(NPAIR)]
            units += [ops_unit(et, c == NCH - 1) for et in range(ND)]
            return units

        # ----- main: pair-outer, chunk-inner; the (p, c) window emits
        # scores+exp for (p, c) and AV+norm for the previous chunk -----
        prev = None        # (p, c, pt_prev) awaiting AV
        pend_out = None    # chunk index awaiting emit_out (pair 3)
        w_nxt = None
        for p in range(NPAIR):
            if p + 1 < NPAIR:
                w_nxt = load_w(p + 1)
                items = [(lambda pp=p + 1, t=t: vproj_t(pp, t))
                         for t in range(NKT)]
                items += [(lambda pp=p + 1, j=j, w=which, ws=w_nxt[which]:
                           qkproj_slice(pp, j, w, ws))
                          for j in range(NCH) for which in range(2)]
            else:
                items = []
            n_items = len(items)
            emitted = 0
            # spread items over this pair's windows (skip window 0 of pair 0,
            # which runs its own projections inline); finish slightly early
            nwin = NCH if p > 0 else NCH - 1
            denom = max(1, nwin * NKT - 6)
            it_count = 0

            for c in range(NCH):
                if p == 0 and c == 0:
                    # inline projections for pair 0, aligned with the
                    # ktile order scores consume them in
                    for i in range(NKT):
                        if i % 4 == 0:
                            qkproj_slice(0, i // 4, 0, w_cur[0])
                            qkproj_slice(0, i // 4, 1, w_cur[1])
                        vproj_t(0, i)
                pt_cur = ptp.tile([128, NKT, 2 * CH], bf16, tag="pt")
                scores_block(p, c, pt_cur)
                if prev is not None:
                    av_norm_block(prev[0], prev[1], prev[2])
                    if prev[0] == NPAIR - 1:
                        pend_out = prev[1]
                if pend_out is not None:
                    for u in emit_out_units(pend_out):
                        u()
                    pend_out = None
                # interleave next-pair projection work
                if items and not (p == 0 and c == 0):
                    it_count += NKT
                    want = min(n_items, (it_count * n_items) // denom)
                    while emitted < want:
                        items[emitted]()
                        emitted += 1
                prev = (p, c, pt_cur)

            while emitted < n_items:
                items[emitted]()
                emitted += 1
            w_cur = w_nxt

        # tail: AV + norm + out-projection for the last chunk
        units = emit_out_units(NCH - 1)
        for u in units[:NPAIR - 1]:   # tps for fb 0..2 (don't need pair 3)
            u()
        av_norm_block(prev[0], prev[1], prev[2])
        for u in units[NPAIR - 1:]:
            u()

        if _DEBUG:
            nc.sync.dma_start(out=dbg_qkT[:], in_=qkT[:])
            nc.sync.dma_start(out=dbg_v[:], in_=v_sb[:])
            nc.sync.dma_start(out=dbg_o[:], in_=o_sb[:])

    nc.compile()
    return nc


def _get_nc():
    if "nc" not in _CACHE:
        _CACHE["nc"] = _build()
    return _CACHE["nc"]


def _bf16(a):
    import ml_dtypes
    return np.ascontiguousarray(
        np.asarray(a, dtype=np.float32).astype(ml_dtypes.bfloat16))


def make_in_map(xT, wqT, wkT, wvT, woT, bq, bk, bv):
    """Pack one core's inputs into the kernel's tiled DRAM layouts.

    xT: [D, S]; wqT/wkT/wvT: [D, FH] (W sections transposed);
    woT: [FH, D] (out_w columns transposed); biases: [FH].
    """
    D, FH, ND, NPAIR, NH, FHA, NCH, CH = (
        _D, _FH, _ND, _NPAIR, _NH, _FHA, _NCH, _CH)
    # augment v with a per-head ones column: wv gets zero columns, bv gets
    # 1.0 entries -> the broadcast bias add installs the ones column, whose
    # AV accumulation yields the softmax denominators for free
    wva = np.zeros((D, FHA), dtype=np.float32)
    bva = np.zeros((1, FHA), dtype=np.float32)
    for h in range(NH):
        wva[:, h * 65:h * 65 + 64] = np.asarray(wvT)[:, h * 64:(h + 1) * 64]
        bva[0, h * 65:h * 65 + 64] = np.asarray(bv)[h * 64:(h + 1) * 64]
        bva[0, h * 65 + 64] = 1.0
    return {
        "xP": _bf16(np.asarray(xT).reshape(ND, 128, NCH, CH).transpose(2, 1, 0, 3)),
        "wq": _bf16(np.asarray(wqT).reshape(ND, 128, NPAIR, 128).transpose(2, 1, 0, 3)),
        "wk": _bf16(np.asarray(wkT).reshape(ND, 128, NPAIR, 128).transpose(2, 1, 0, 3)),
        "wv": _bf16(wva.reshape(ND, 128, FHA).transpose(1, 0, 2)),
        "wo": _bf16(np.asarray(woT).reshape(NPAIR, 128, D).transpose(1, 0, 2)),
        "bq": np.ascontiguousarray(
            np.asarray(bq, dtype=np.float32).reshape(NPAIR, 128).T),
        "bk": np.ascontiguousarray(
            np.asarray(bk, dtype=np.float32).reshape(NPAIR, 128).T),
        "bv": _bf16(bva),
        "idn": _bf16(np.eye(128)),
    }


def unpack_out(outp_tiled):
    """[NCH, ND, 128, CH] tiled partial -> [D, S] float32."""
    a = np.asarray(outp_tiled, dtype=np.float32)
    return a.transpose(1, 2, 0, 3).reshape(_D, _S)


def _shard_inputs(x, in_proj_weight, in_proj_bias, out_w):
    w = np.asarray(in_proj_weight)
    b = np.asarray(in_proj_bias)
    ow = np.asarray(out_w)
    in_maps = []
    for c in range(_NCORES):
        bi, g = divmod(c, 2)
        sl = slice(g * _FH, (g + 1) * _FH)
        in_maps.append(make_in_map(
            xT=np.asarray(x[bi]).T,
            wqT=w[0 * _D:1 * _D][sl].T,
            wkT=w[1 * _D:2 * _D][sl].T,
            wvT=w[2 * _D:3 * _D][sl].T,
            woT=ow[:, sl].T,
            bq=b[0 * _D:1 * _D][sl],
            bk=b[1 * _D:2 * _D][sl],
            bv=b[2 * _D:3 * _D][sl],
        ))
    return in_maps


LAST_RESULTS = None


def kernel(x, in_proj_weight, in_proj_bias, out_w, out_b):
    global LAST_RESULTS
    from concourse.bass_utils import run_bass_kernel_spmd
    import os

    nc = _get_nc()
    in_maps = _shard_inputs(x, in_proj_weight, in_proj_bias, out_w)
    trace = os.environ.get("BASS_TRACE", "0") not in ("", "0")
    res = run_bass_kernel_spmd(
        nc, in_maps, core_ids=list(range(_NCORES)), trace=trace
    )
    LAST_RESULTS = res
    out_b = np.asarray(out_b, dtype=np.float32)
    out = np.empty((_B, _S, _D), dtype=np.float32)
    for b in range(_B):
        part = (unpack_out(res.results[2 * b]["outp"])
                + unpack_out(res.results[2 * b + 1]["outp"]))
        out[b] = part.T + out_b
    return out


# revision 24
# speedup vs baseline: 1.5698x; 1.0265x over previous
"""Multi-head self-attention (B=4, S=2048, D=1024, H=16) on 8 NeuronCores.

Sharding: data-parallel over batch (4 groups) x tensor-parallel over heads
(2 groups of 8 heads).  Core c handles batch b=c//2, head-group g=c%2.
Each core computes its 8 heads' attention plus a partial out-projection;
the host sums the two partials per batch, transposes, adds out_b.

v2 design (vs v1): everything in bf16 (plenty of margin vs the 2e-2 gate),
no DRAM staging, and a "flipped" AV matmul that halves the PE rows:

  - scores^T per head-pair via row-packed K=64 matmuls, psum tile
    [128 keys, 2*CH] holds both heads; ONE exp per ktile ([128, 1024]
    activation, scale=1/8 folded) -> pt [keys, q] bf16 in SBUF
  - AV flipped: stationary = pt q-block [128 keys, 128 q], moving = v_aug
    [128 keys, 65] (64 v dims + ones column) -> psum [128 q, 65] per
    (qblock, head), accumulated over the 16 ktiles.  65 moving rows per
    ktile instead of 128 q rows: ~2x fewer PE cycles than v1's AV.  The
    ones column (installed by a broadcast bias add on the v projection)
    lands the softmax denominator in psum column 64, per PARTITION
    (= per q), so normalization is a native tensor_scalar multiply.
  - PSUM accumulation groups must run ONE AT A TIME per psum bank
    (a start=True while another group is open in the same bank resets the
    bank - verified on hw).  So AV for chunk (c) runs as a post-pass
    (one (qblock, head) group after another) in the NEXT chunk's window,
    with pt double-buffered.
  - o comes out [q, feat]; out-projection needs o^T, done with cheap PE
    transposes (128 rows each) through PSUM.
  - schedule: pair-outer; pair p+1's q/k/v projections and chunk c-1's
    out-projection interleave into pair p's ACT-bound attention windows.
    ACT does only the 256 exps (~266us); PE ~280us; DVE does all
    PSUM->SBUF moves + bias/normalize (~110us).  gpsimd cannot read PSUM
    (walrus codegen fails) so DVE carries the copies.
"""

import numpy as np

_B, _S, _D, _H = 4, 2048, 1024, 16
_FH = 512        # local feature dims per core (8 heads x 64)
_ND = _D // 128  # contraction tiles
_NPAIR = 4       # head pairs (2 heads of 64 -> 128 features)
_NKT = _S // 128 # key tiles
_CH = 512        # q chunk
_NCH = _S // _CH
_NQB = _CH // 128
_NH = 8          # local heads
_FHA = _NH * 65  # v width incl. per-head ones column
_NCORES = 8

_CACHE = {}
_DEBUG = False  # adds qkT/v/o debug outputs to the kernel


def _build():
    import concourse.bass as bass
    import concourse.bacc as bacc
    import concourse.tile as tile
    import concourse.mybir as mybir
    from contextlib import ExitStack

    f32 = mybir.dt.float32
    bf16 = mybir.dt.bfloat16
    Exp = mybir.ActivationFunctionType.Exp
    D, S, FH, ND, NPAIR, NKT, CH, NCH, NQB, FHA = (
        _D, _S, _FH, _ND, _NPAIR, _NKT, _CH, _NCH, _NQB, _FHA)

    nc = bacc.Bacc("TRN2", target_bir_lowering=False, debug=False)

    xP_d = nc.dram_tensor("xP", [NCH, 128, ND, CH], bf16, kind="ExternalInput")
    wq_d = nc.dram_tensor("wq", [NPAIR, 128, ND, 128], bf16, kind="ExternalInput")
    wk_d = nc.dram_tensor("wk", [NPAIR, 128, ND, 128], bf16, kind="ExternalInput")
    wv_d = nc.dram_tensor("wv", [128, ND, FHA], bf16, kind="ExternalInput")
    wo_d = nc.dram_tensor("wo", [128, NPAIR, D], bf16, kind="ExternalInput")
    bq_d = nc.dram_tensor("bq", [128, NPAIR], f32, kind="ExternalInput")
    bk_d = nc.dram_tensor("bk", [128, NPAIR], f32, kind="ExternalInput")
    bv_d = nc.dram_tensor("bv", [1, FHA], bf16, kind="ExternalInput")
    idn_d = nc.dram_tensor("idn", [128, 128], bf16, kind="ExternalInput")
    outp_d = nc.dram_tensor("outp", [NCH, ND, 128, CH], bf16, kind="ExternalOutput")
    if _DEBUG:
        dbg_qkT = nc.dram_tensor("dbg_qkT", [128, NPAIR, 2, S], bf16, kind="ExternalOutput")
        dbg_v = nc.dram_tensor("dbg_v", [128, NKT, FHA], bf16, kind="ExternalOutput")
        dbg_o = nc.dram_tensor("dbg_o", [128, S // 128, FH], bf16, kind="ExternalOutput")

    with tile.TileContext(nc) as tc, ExitStack() as top:
        consts = top.enter_context(tc.tile_pool(name="consts", bufs=1))
        ps = top.enter_context(tc.tile_pool(name="ps", bufs=2, space="PSUM"))
        big = top.enter_context(tc.tile_pool(name="big", bufs=1))
        ptp = top.enter_context(tc.tile_pool(name="ptp", bufs=2))
        wst = top.enter_context(tc.tile_pool(name="wst", bufs=2))
        otp = top.enter_context(tc.tile_pool(name="otp", bufs=2))
        stp = top.enter_context(tc.tile_pool(name="stp", bufs=3))
        rcpp = top.enter_context(tc.tile_pool(name="rcpp", bufs=2))

        xT_sb = big.tile([128, ND, S], bf16)
        qkT = big.tile([128, NPAIR, 2, S], bf16)  # [feat%128, pair, q/k, t]
        v_sb = big.tile([128, NKT, FHA], bf16)    # [token%128, ktile, head*65]
        o_sb = big.tile([128, S // 128, FH], bf16)  # [q%128, qblock, feat]
        wv_sb = big.tile([128, ND, FHA], bf16)
        wo_sb = big.tile([128, NPAIR, D], bf16)

        def load_w(p):
            wq_sb = wst.tile([128, ND, 128], bf16, tag="wq")
            nc.sync.dma_start(out=wq_sb, in_=wq_d[p])
            wk_sb = wst.tile([128, ND, 128], bf16, tag="wk")
            nc.sync.dma_start(out=wk_sb, in_=wk_d[p])
            return wq_sb, wk_sb

        # DMA order tuned for the warmup critical path: the first qkproj
        # slice needs x slice 0 + wq0/wk0; the first vproj needs wv
        nc.sync.dma_start(out=xT_sb[:, :, 0:CH], in_=xP_d[0])
        w_cur = load_w(0)
        nc.sync.dma_start(out=wv_sb, in_=wv_d[:])
        bqk_sb = consts.tile([128, 2 * NPAIR], f32)
        nc.sync.dma_start(out=bqk_sb[:, 0:NPAIR], in_=bq_d[:])
        nc.sync.dma_start(out=bqk_sb[:, NPAIR:2 * NPAIR], in_=bk_d[:])
        # v bias broadcast to all partitions (includes the 1.0 ones-column
        # entries that seed the softmax-denominator trick)
        bvb_sb = consts.tile([128, FHA], bf16)
        nc.sync.dma_start(out=bvb_sb, in_=bv_d[:].to_broadcast([128, FHA]))
        for ts in range(1, NCH):
            nc.sync.dma_start(
                out=xT_sb[:, :, ts * CH:(ts + 1) * CH], in_=xP_d[ts])
        idn_sb = consts.tile([128, 128], bf16)
        nc.sync.dma_start(out=idn_sb, in_=idn_d[:])
        nc.sync.dma_start(out=wo_sb, in_=wo_d[:])
        # dummy exp so the ACT table set loads during the ramp
        warm = consts.tile([1, 8], f32)
        nc.vector.memset(warm, 0.0)
        nc.scalar.activation(out=warm, in_=warm, func=Exp)

        def qkproj_slice(p, j, which, w_sb):
            pps = ps.tile([128, CH], f32, tag="mix")
            for d in range(ND):
                nc.tensor.matmul(
                    pps,
                    lhsT=w_sb[:, d, :],
                    rhs=xT_sb[:, d, j * CH:(j + 1) * CH],
                    start=(d == 0),
                    stop=(d == ND - 1),
                )
            nc.vector.tensor_scalar_add(
                out=qkT[:, p, which, j * CH:(j + 1) * CH],
                in0=pps,
                scalar1=bqk_sb[:, which * NPAIR + p:which * NPAIR + p + 1],
            )

        def vproj_t(p, t):
            vps = ps.tile([128, 130], f32, tag="mix")
            for d in range(ND):
                nc.tensor.matmul(
                    vps,
                    lhsT=xT_sb[:, d, t * 128:(t + 1) * 128],
                    rhs=wv_sb[:, d, p * 130:(p + 1) * 130],
                    start=(d == 0),
                    stop=(d == ND - 1),
                )
            nc.vector.tensor_add(
                out=v_sb[:, t, p * 130:(p + 1) * 130],
                in0=vps,
                in1=bvb_sb[:, p * 130:(p + 1) * 130],
            )

        def score_unit(p, c, i, pt_cur):
            sAB = ps.tile([128, 2 * CH], f32, tag="sab")
            nc.tensor.matmul(
                sAB[:, 0:CH],
                lhsT=qkT[0:64, p, 1, i * 128:(i + 1) * 128],
                rhs=qkT[0:64, p, 0, c * CH:(c + 1) * CH],
                start=True, stop=True,
                tile_position=(0, 0),
            )
            nc.tensor.matmul(
                sAB[:, CH:2 * CH],
                lhsT=qkT[64:128, p, 1, i * 128:(i + 1) * 128],
                rhs=qkT[64:128, p, 0, c * CH:(c + 1) * CH],
                start=True, stop=True,
                tile_position=(64, 0),
            )
            nc.scalar.activation(
                out=pt_cur[:, i, :], in_=sAB, func=Exp, scale=0.125)

        def av_ktile(p, av_t, half, i, pt_cur, first, last):
            """AV matmuls for one psum bank (av01 or av23) at ktile i.

            The bank runs ONE accumulation context per chunk: start=True
            only on the bank's first write (resets the bank's
            written-bitmap; untouched regions then store on first touch,
            accumulate after -- verified on hw), stop on its last.
            """
            for qbl in range(2):
                qb = half * 2 + qbl
                for h in range(2):
                    nc.tensor.matmul(
                        av_t[half][:, qbl, h * 65:(h + 1) * 65],
                        lhsT=pt_cur[:, i, h * CH + qb * 128:
                                    h * CH + (qb + 1) * 128],
                        rhs=v_sb[:, i, p * 130 + h * 65:
                                 p * 130 + (h + 1) * 65],
                        start=(first and qbl == 0 and h == 0),
                        stop=(last and qbl == 1 and h == 1),
                        skip_group_check=True,
                    )

        def norm_chunk(p, c, av_t, rcp_t):
            for half in range(2):
                for h in range(2):
                    nc.vector.reciprocal_approx_fast(
                        out=rcp_t[:, half, 2 * h:2 * h + 1],
                        in_=av_t[half][:, 0:1, 64 + 65 * h:65 + 65 * h],
                    )
                    nc.vector.reciprocal_approx_fast(
                        out=rcp_t[:, half, 2 * h + 1:2 * h + 2],
                        in_=av_t[half][:, 1:2, 64 + 65 * h:65 + 65 * h],
                    )
                for qbl in range(2):
                    qb = half * 2 + qbl
                    for h in range(2):
                        nc.vector.tensor_scalar_mul(
                            out=o_sb[:, c * NQB + qb,
                                     p * 128 + h * 64:p * 128 + (h + 1) * 64],
                            in0=av_t[half][:, qbl, h * 65:h * 65 + 64],
                            scalar1=rcp_t[:, half,
                                          2 * h + qbl:2 * h + qbl + 1],
                        )

        def emit_out_units(c):
            """Transposes + out-projection for chunk c, as thunks.

            tps units for fb<3 depend only on pairs 0-2 (whose chunk-c
            norms ran long ago); fb==3 waits on pair 3's norm.
            """
            oT = otp.tile([128, NPAIR, CH], bf16, tag="ot")

            def tps_unit(fb):
                def go():
                    tps = ps.tile([128, NQB, 128], bf16, tag="mix")
                    for qb in range(NQB):
                        nc.tensor.transpose(
                            out=tps[:, qb, :],
                            in_=o_sb[:, c * NQB + qb, fb * 128:(fb + 1) * 128],
                            identity=idn_sb,
                        )
                    nc.vector.tensor_copy(out=oT[:, fb, :], in_=tps)
                return go

            def ops_unit(et, on_act):
                def go():
                    ops = ps.tile([128, CH], f32, tag="mix")
                    for pb in range(NPAIR):
                        nc.tensor.matmul(
                            ops,
                            lhsT=wo_sb[:, pb, et * 128:(et + 1) * 128],
                            rhs=oT[:, pb, :],
                            start=(pb == 0),
                            stop=(pb == NPAIR - 1),
                        )
                    st = stp.tile([128, CH], bf16, tag="st")
                    if on_act:
                        nc.scalar.copy(out=st, in_=ops)
                    else:
                        nc.vector.tensor_copy(out=st, in_=ops)
                    nc.sync.dma_start(out=outp_d[c, et], in_=st)
                return go

            units = [tps_unit(fb) for fb in range(NPAIR)]
            # in the tail (last chunk) ACT is idle: alternate the psum->sbuf
            # copies between ACT and DVE so neither serializes the drain
            units += [ops_unit(et, c == NCH - 1 and et % 2 == 0)
                      for et in range(ND)]
            return units

        # ----- main: pair-outer, chunk-inner.  Per ktile: scores+exp for
        # ktile i, AV for ktile i-1 (bank av01) and i-2 (bank av23) -- the
        # lag keeps the in-order PE from blocking on the just-issued exp.
        # Projection / out-projection filler work is cost-paced between
        # ktiles so the PE never starves while pacing behind ACT. -----
        pend_out = None    # chunk index awaiting emit_out (pair 3)
        carry = []         # (thunk, deadline-ktile) for window (3, 0)
        w_nxt = None
        for p in range(NPAIR):
            if p + 1 < NPAIR:
                w_nxt = load_w(p + 1)
                items = [(500, (lambda pp=p + 1, t=t: vproj_t(pp, t)))
                         for t in range(NKT)]
                items += [(1740, (lambda pp=p + 1, j=j, w=which,
                                  ws=w_nxt[which]:
                           qkproj_slice(pp, j, w, ws)))
                          for j in range(NCH) for which in range(2)]
                if p == 2:
                    # window (3,0) has no filler work of its own: carry
                    # pair 3's last qkproj slices there, each emitted
                    # before the ktile that first consumes its k-slice
                    carry = [(items[20][1], 4), (items[21][1], 6),
                             (items[22][1], 8), (items[23][1], 10)]
                    items = items[:20]
            else:
                items = []
            n_items = len(items)
            emitted = 0
            nwin = NCH if p > 0 else NCH - 1
            denom = max(1, nwin * NKT - 6)
            it_count = 0

            for c in range(NCH):
                # fillers: (cost_ns, thunk) of PE work to spread between
                # the score units
                fillers = []
                if pend_out is not None:
                    fillers += [(220, u) if k < NPAIR else (870, u)
                                for k, u in enumerate(emit_out_units(pend_out))]
                    pend_out = None
                if items and not (p == 0 and c == 0):
                    it_count += NKT
                    want = min(n_items, (it_count * n_items) // denom)
                    while emitted < want:
                        fillers.append(items[emitted])
                        emitted += 1

                pt_cur = ptp.tile([128, NKT, 2 * CH], bf16, tag="pt")
                av_t = [ps.tile([128, 2, 130], f32, tag="av", name=f"av{h}")
                        for h in range(2)]
                rcp_t = rcpp.tile([128, 2, 4], f32, tag="rcp")
                total = sum(cn for cn, _ in fillers)
                spent = 0
                for i in range(NKT):
                    if p == 0 and c == 0:
                        # inline projections for pair 0, aligned with the
                        # ktile order scores consume them in
                        if i % 4 == 0:
                            qkproj_slice(0, i // 4, 0, w_cur[0])
                            qkproj_slice(0, i // 4, 1, w_cur[1])
                        vproj_t(0, i)
                    if p == NPAIR - 1 and c == 0:
                        while carry and carry[0][1] <= i:
                            carry.pop(0)[0]()
                    score_unit(p, c, i, pt_cur)
                    if i >= 2:
                        av_ktile(p, av_t, 0, i - 2, pt_cur,
                                 first=(i == 2), last=False)
                    if i >= 3:
                        av_ktile(p, av_t, 1, i - 3, pt_cur,
                                 first=(i == 3), last=False)
                    if i >= 1 and fillers:
                        target = (total * i) // (NKT - 1)
                        while fillers and spent < target:
                            cn, f = fillers.pop(0)
                            f()
                            spent += cn
                for k in (NKT - 2, NKT - 1):
                    av_ktile(p, av_t, 0, k, pt_cur, first=False,
                             last=(k == NKT - 1))
                for k in (NKT - 3, NKT - 2, NKT - 1):
                    av_ktile(p, av_t, 1, k, pt_cur, first=False,
                             last=(k == NKT - 1))
                for _, f in fillers:
                    f()
                norm_chunk(p, c, av_t, rcp_t)
                if p == NPAIR - 1 and c < NCH - 1:
                    pend_out = c

            while emitted < n_items:
                items[emitted][1]()
                emitted += 1
            w_cur = w_nxt

        # tail: out-projection for the last chunk
        for u in emit_out_units(NCH - 1):
            u()

        if _DEBUG:
            nc.sync.dma_start(out=dbg_qkT[:], in_=qkT[:])
            nc.sync.dma_start(out=dbg_v[:], in_=v_sb[:])
            nc.sync.dma_start(out=dbg_o[:], in_=o_sb[:])

    nc.compile()
    return nc


def _get_nc():
    if "nc" not in _CACHE:
        _CACHE["nc"] = _build()
    return _CACHE["nc"]


def _bf16(a):
    import ml_dtypes
    return np.ascontiguousarray(
        np.asarray(a, dtype=np.float32).astype(ml_dtypes.bfloat16))


def make_in_map(xT, wqT, wkT, wvT, woT, bq, bk, bv):
    """Pack one core's inputs into the kernel's tiled DRAM layouts.

    xT: [D, S]; wqT/wkT/wvT: [D, FH] (W sections transposed);
    woT: [FH, D] (out_w columns transposed); biases: [FH].
    """
    D, FH, ND, NPAIR, NH, FHA, NCH, CH = (
        _D, _FH, _ND, _NPAIR, _NH, _FHA, _NCH, _CH)
    # augment v with a per-head ones column: wv gets zero columns, bv gets
    # 1.0 entries -> the broadcast bias add installs the ones column, whose
    # AV accumulation yields the softmax denominators for free
    wva = np.zeros((D, FHA), dtype=np.float32)
    bva = np.zeros((1, FHA), dtype=np.float32)
    for h in range(NH):
        wva[:, h * 65:h * 65 + 64] = np.asarray(wvT)[:, h * 64:(h + 1) * 64]
        bva[0, h * 65:h * 65 + 64] = np.asarray(bv)[h * 64:(h + 1) * 64]
        bva[0, h * 65 + 64] = 1.0
    return {
        "xP": _bf16(np.asarray(xT).reshape(ND, 128, NCH, CH).transpose(2, 1, 0, 3)),
        "wq": _bf16(np.asarray(wqT).reshape(ND, 128, NPAIR, 128).transpose(2, 1, 0, 3)),
        "wk": _bf16(np.asarray(wkT).reshape(ND, 128, NPAIR, 128).transpose(2, 1, 0, 3)),
        "wv": _bf16(wva.reshape(ND, 128, FHA).transpose(1, 0, 2)),
        "wo": _bf16(np.asarray(woT).reshape(NPAIR, 128, D).transpose(1, 0, 2)),
        "bq": np.ascontiguousarray(
            np.asarray(bq, dtype=np.float32).reshape(NPAIR, 128).T),
        "bk": np.ascontiguousarray(
            np.asarray(bk, dtype=np.float32).reshape(NPAIR, 128).T),
        "bv": _bf16(bva),
        "idn": _bf16(np.eye(128)),
    }


def unpack_out(outp_tiled):
    """[NCH, ND, 128, CH] tiled partial -> [D, S] float32."""
    a = np.asarray(outp_tiled, dtype=np.float32)
    return a.transpose(1, 2, 0, 3).reshape(_D, _S)


def _shard_inputs(x, in_proj_weight, in_proj_bias, out_w):
    w = np.asarray(in_proj_weight)
    b = np.asarray(in_proj_bias)
    ow = np.asarray(out_w)
    in_maps = []
    for c in range(_NCORES):
        bi, g = divmod(c, 2)
        sl = slice(g * _FH, (g + 1) * _FH)
        in_maps.append(make_in_map(
            xT=np.asarray(x[bi]).T,
            wqT=w[0 * _D:1 * _D][sl].T,
            wkT=w[1 * _D:2 * _D][sl].T,
            wvT=w[2 * _D:3 * _D][sl].T,
            woT=ow[:, sl].T,
            bq=b[0 * _D:1 * _D][sl],
            bk=b[1 * _D:2 * _D][sl],
            bv=b[2 * _D:3 * _D][sl],
        ))
    return in_maps


LAST_RESULTS = None


def kernel(x, in_proj_weight, in_proj_bias, out_w, out_b):
    global LAST_RESULTS
    from concourse.bass_utils import run_bass_kernel_spmd
    import os

    nc = _get_nc()
    in_maps = _shard_inputs(x, in_proj_weight, in_proj_bias, out_w)
    trace = os.environ.get("BASS_TRACE", "0") not in ("", "0")
    res = run_bass_kernel_spmd(
        nc, in_maps, core_ids=list(range(_NCORES)), trace=trace
    )
    LAST_RESULTS = res
    out_b = np.asarray(out_b, dtype=np.float32)
    out = np.empty((_B, _S, _D), dtype=np.float32)
    for b in range(_B):
        part = (unpack_out(res.results[2 * b]["outp"])
                + unpack_out(res.results[2 * b + 1]["outp"]))
        out[b] = part.T + out_b
    return out


# revision 28
# speedup vs baseline: 1.5942x; 1.0155x over previous
"""Multi-head self-attention (B=4, S=2048, D=1024, H=16) on 8 NeuronCores.

Sharding: data-parallel over batch (4 groups) x tensor-parallel over heads
(2 groups of 8 heads).  Core c handles batch b=c//2, head-group g=c%2.
Each core computes its 8 heads' attention plus a partial out-projection;
the host sums the two partials per batch, transposes, adds out_b.

v2 design (vs v1): everything in bf16 (plenty of margin vs the 2e-2 gate),
no DRAM staging, and a "flipped" AV matmul that halves the PE rows:

  - scores^T per head-pair via row-packed K=64 matmuls, psum tile
    [128 keys, 2*CH] holds both heads; ONE exp per ktile ([128, 1024]
    activation, scale=1/8 folded) -> pt [keys, q] bf16 in SBUF
  - AV flipped: stationary = pt q-block [128 keys, 128 q], moving = v_aug
    [128 keys, 65] (64 v dims + ones column) -> psum [128 q, 65] per
    (qblock, head), accumulated over the 16 ktiles.  65 moving rows per
    ktile instead of 128 q rows: ~2x fewer PE cycles than v1's AV.  The
    ones column (installed by a broadcast bias add on the v projection)
    lands the softmax denominator in psum column 64, per PARTITION
    (= per q), so normalization is a native tensor_scalar multiply.
  - PSUM accumulation groups must run ONE AT A TIME per psum bank
    (a start=True while another group is open in the same bank resets the
    bank - verified on hw).  So AV for chunk (c) runs as a post-pass
    (one (qblock, head) group after another) in the NEXT chunk's window,
    with pt double-buffered.
  - o comes out [q, feat]; out-projection needs o^T, done with cheap PE
    transposes (128 rows each) through PSUM.
  - schedule: pair-outer; pair p+1's q/k/v projections and chunk c-1's
    out-projection interleave into pair p's ACT-bound attention windows.
    ACT does only the 256 exps (~266us); PE ~280us; DVE does all
    PSUM->SBUF moves + bias/normalize (~110us).  gpsimd cannot read PSUM
    (walrus codegen fails) so DVE carries the copies.
"""

import numpy as np

_B, _S, _D, _H = 4, 2048, 1024, 16
_FH = 512        # local feature dims per core (8 heads x 64)
_ND = _D // 128  # contraction tiles
_NPAIR = 4       # head pairs (2 heads of 64 -> 128 features)
_NKT = _S // 128 # key tiles
_CH = 512        # q chunk
_NCH = _S // _CH
_NQB = _CH // 128
_NH = 8          # local heads
_FHA = _NH * 65  # v width incl. per-head ones column
_NCORES = 8

_CACHE = {}
_DEBUG = False  # adds qkT/v/o debug outputs to the kernel


def _build():
    import concourse.bass as bass
    import concourse.bacc as bacc
    import concourse.tile as tile
    import concourse.mybir as mybir
    from contextlib import ExitStack

    f32 = mybir.dt.float32
    bf16 = mybir.dt.bfloat16
    Exp = mybir.ActivationFunctionType.Exp
    D, S, FH, ND, NPAIR, NKT, CH, NCH, NQB, FHA = (
        _D, _S, _FH, _ND, _NPAIR, _NKT, _CH, _NCH, _NQB, _FHA)

    nc = bacc.Bacc("TRN2", target_bir_lowering=False, debug=False)

    xP_d = nc.dram_tensor("xP", [NCH, 128, ND, CH], bf16, kind="ExternalInput")
    wq_d = nc.dram_tensor("wq", [NPAIR, 128, ND, 128], bf16, kind="ExternalInput")
    wk_d = nc.dram_tensor("wk", [NPAIR, 128, ND, 128], bf16, kind="ExternalInput")
    wv_d = nc.dram_tensor("wv", [128, ND, FHA], bf16, kind="ExternalInput")
    wo_d = nc.dram_tensor("wo", [128, NPAIR, D], bf16, kind="ExternalInput")
    bq_d = nc.dram_tensor("bq", [128, NPAIR], f32, kind="ExternalInput")
    bk_d = nc.dram_tensor("bk", [128, NPAIR], f32, kind="ExternalInput")
    bv_d = nc.dram_tensor("bv", [1, FHA], bf16, kind="ExternalInput")
    idn_d = nc.dram_tensor("idn", [128, 128], bf16, kind="ExternalInput")
    outp_d = nc.dram_tensor("outp", [NCH, ND, 128, CH], bf16, kind="ExternalOutput")
    if _DEBUG:
        dbg_qkT = nc.dram_tensor("dbg_qkT", [128, NPAIR, 2, S], bf16, kind="ExternalOutput")
        dbg_v = nc.dram_tensor("dbg_v", [128, NKT, FHA], bf16, kind="ExternalOutput")
        dbg_o = nc.dram_tensor("dbg_o", [128, S // 128, FH], bf16, kind="ExternalOutput")

    with tile.TileContext(nc) as tc, ExitStack() as top:
        consts = top.enter_context(tc.tile_pool(name="consts", bufs=1))
        ps = top.enter_context(tc.tile_pool(name="ps", bufs=2, space="PSUM"))
        big = top.enter_context(tc.tile_pool(name="big", bufs=1))
        ptp = top.enter_context(tc.tile_pool(name="ptp", bufs=2))
        wst = top.enter_context(tc.tile_pool(name="wst", bufs=2))
        otp = top.enter_context(tc.tile_pool(name="otp", bufs=2))
        stp = top.enter_context(tc.tile_pool(name="stp", bufs=3))
        rcpp = top.enter_context(tc.tile_pool(name="rcpp", bufs=2))

        xT_sb = big.tile([128, ND, S], bf16)
        qkT = big.tile([128, NPAIR, 2, S], bf16)  # [feat%128, pair, q/k, t]
        v_sb = big.tile([128, NKT, FHA], bf16)    # [token%128, ktile, head*65]
        o_sb = big.tile([128, S // 128, FH], bf16)  # [q%128, qblock, feat]
        wv_sb = big.tile([128, ND, FHA], bf16)
        wo_sb = big.tile([128, NPAIR, D], bf16)

        def load_w(p):
            wq_sb = wst.tile([128, ND, 128], bf16, tag="wq")
            nc.sync.dma_start(out=wq_sb, in_=wq_d[p])
            wk_sb = wst.tile([128, ND, 128], bf16, tag="wk")
            nc.sync.dma_start(out=wk_sb, in_=wk_d[p])
            return wq_sb, wk_sb

        # DMA order tuned for the warmup critical path: the first qkproj
        # half-slice needs the first half of x slice 0 + wq0/wk0; the
        # first vproj needs wv
        w_cur = load_w(0)
        nc.sync.dma_start(out=xT_sb[:, :, 0:CH // 2], in_=xP_d[0][:, :, 0:CH // 2])
        nc.sync.dma_start(out=xT_sb[:, :, CH // 2:CH], in_=xP_d[0][:, :, CH // 2:CH])
        nc.sync.dma_start(out=wv_sb, in_=wv_d[:])
        bqk_sb = consts.tile([128, 2 * NPAIR], f32)
        nc.sync.dma_start(out=bqk_sb[:, 0:NPAIR], in_=bq_d[:])
        nc.sync.dma_start(out=bqk_sb[:, NPAIR:2 * NPAIR], in_=bk_d[:])
        # v bias broadcast to all partitions (includes the 1.0 ones-column
        # entries that seed the softmax-denominator trick)
        bvb_sb = consts.tile([128, FHA], bf16)
        nc.sync.dma_start(out=bvb_sb, in_=bv_d[:].to_broadcast([128, FHA]))
        for ts in range(1, NCH):
            nc.sync.dma_start(
                out=xT_sb[:, :, ts * CH:(ts + 1) * CH], in_=xP_d[ts])
        idn_sb = consts.tile([128, 128], bf16)
        nc.sync.dma_start(out=idn_sb, in_=idn_d[:])
        nc.sync.dma_start(out=wo_sb, in_=wo_d[:])
        # dummy exp so the ACT table set loads during the ramp
        warm = consts.tile([1, 8], f32)
        nc.vector.memset(warm, 0.0)
        nc.scalar.activation(out=warm, in_=warm, func=Exp)

        def qkproj_slice(p, j, which, w_sb, halves=1):
            pps = ps.tile([128, CH], f32, tag="mix")
            hw_ = CH // halves
            for hf in range(halves):
                for d in range(ND):
                    nc.tensor.matmul(
                        pps[:, hf * hw_:(hf + 1) * hw_],
                        lhsT=w_sb[:, d, :],
                        rhs=xT_sb[:, d, j * CH + hf * hw_:
                                  j * CH + (hf + 1) * hw_],
                        start=(d == 0),
                        stop=(d == ND - 1),
                    )
            nc.vector.tensor_scalar_add(
                out=qkT[:, p, which, j * CH:(j + 1) * CH],
                in0=pps,
                scalar1=bqk_sb[:, which * NPAIR + p:which * NPAIR + p + 1],
            )

        def vproj_t(p, t):
            vps = ps.tile([128, 130], f32, tag="mix")
            for d in range(ND):
                nc.tensor.matmul(
                    vps,
                    lhsT=xT_sb[:, d, t * 128:(t + 1) * 128],
                    rhs=wv_sb[:, d, p * 130:(p + 1) * 130],
                    start=(d == 0),
                    stop=(d == ND - 1),
                )
            nc.vector.tensor_add(
                out=v_sb[:, t, p * 130:(p + 1) * 130],
                in0=vps,
                in1=bvb_sb[:, p * 130:(p + 1) * 130],
            )

        def score_unit(p, c, i, pt_cur):
            sAB = ps.tile([128, 2 * CH], f32, tag="sab")
            nc.tensor.matmul(
                sAB[:, 0:CH],
                lhsT=qkT[0:64, p, 1, i * 128:(i + 1) * 128],
                rhs=qkT[0:64, p, 0, c * CH:(c + 1) * CH],
                start=True, stop=True,
                tile_position=(0, 0),
            )
            nc.tensor.matmul(
                sAB[:, CH:2 * CH],
                lhsT=qkT[64:128, p, 1, i * 128:(i + 1) * 128],
                rhs=qkT[64:128, p, 0, c * CH:(c + 1) * CH],
                start=True, stop=True,
                tile_position=(64, 0),
            )
            nc.scalar.activation(
                out=pt_cur[:, i, :], in_=sAB, func=Exp, scale=0.125)

        def av_ktile(p, av_t, half, i, pt_cur, first, last):
            """AV matmuls for one psum bank (av01 or av23) at ktile i.

            The bank runs ONE accumulation context per chunk: start=True
            only on the bank's first write (resets the bank's
            written-bitmap; untouched regions then store on first touch,
            accumulate after -- verified on hw), stop on its last.
            """
            for qbl in range(2):
                qb = half * 2 + qbl
                for h in range(2):
                    nc.tensor.matmul(
                        av_t[half][:, qbl, h * 65:(h + 1) * 65],
                        lhsT=pt_cur[:, i, h * CH + qb * 128:
                                    h * CH + (qb + 1) * 128],
                        rhs=v_sb[:, i, p * 130 + h * 65:
                                 p * 130 + (h + 1) * 65],
                        start=(first and qbl == 0 and h == 0),
                        stop=(last and qbl == 1 and h == 1),
                        skip_group_check=True,
                    )

        def norm_chunk(p, c, av_t, rcp_t):
            for half in range(2):
                for h in range(2):
                    nc.vector.reciprocal_approx_fast(
                        out=rcp_t[:, half, 2 * h:2 * h + 1],
                        in_=av_t[half][:, 0:1, 64 + 65 * h:65 + 65 * h],
                    )
                    nc.vector.reciprocal_approx_fast(
                        out=rcp_t[:, half, 2 * h + 1:2 * h + 2],
                        in_=av_t[half][:, 1:2, 64 + 65 * h:65 + 65 * h],
                    )
                for qbl in range(2):
                    qb = half * 2 + qbl
                    for h in range(2):
                        nc.vector.tensor_scalar_mul(
                            out=o_sb[:, c * NQB + qb,
                                     p * 128 + h * 64:p * 128 + (h + 1) * 64],
                            in0=av_t[half][:, qbl, h * 65:h * 65 + 64],
                            scalar1=rcp_t[:, half,
                                          2 * h + qbl:2 * h + qbl + 1],
                        )

        def emit_out_units(c):
            """Transposes + out-projection for chunk c, as thunks.

            tps units for fb<3 depend only on pairs 0-2 (whose chunk-c
            norms ran long ago); fb==3 waits on pair 3's norm.
            """
            oT = otp.tile([128, NPAIR, CH], bf16, tag="ot")

            def tps_unit(fb):
                def go():
                    tps = ps.tile([128, NQB, 128], bf16, tag="mix")
                    for qb in range(NQB):
                        nc.tensor.transpose(
                            out=tps[:, qb, :],
                            in_=o_sb[:, c * NQB + qb, fb * 128:(fb + 1) * 128],
                            identity=idn_sb,
                        )
                    nc.vector.tensor_copy(out=oT[:, fb, :], in_=tps)
                return go

            def ops_unit(et, on_act):
                def go():
                    ops = ps.tile([128, CH], f32, tag="mix")
                    for pb in range(NPAIR):
                        nc.tensor.matmul(
                            ops,
                            lhsT=wo_sb[:, pb, et * 128:(et + 1) * 128],
                            rhs=oT[:, pb, :],
                            start=(pb == 0),
                            stop=(pb == NPAIR - 1),
                        )
                    st = stp.tile([128, CH], bf16, tag="st")
                    if on_act:
                        nc.scalar.copy(out=st, in_=ops)
                    else:
                        nc.vector.tensor_copy(out=st, in_=ops)
                    nc.sync.dma_start(out=outp_d[c, et], in_=st)
                return go

            units = [tps_unit(fb) for fb in range(NPAIR)]
            # toward the tail ACT gains slack: alternate the psum->sbuf
            # copies between ACT and DVE so the DVE queue (which also
            # carries the final norm) doesn't serialize the drain
            units += [ops_unit(et, c >= NCH - 2 and et % 2 == 0)
                      for et in range(ND)]
            return units

        # ----- main: pair-outer, chunk-inner.  Per ktile: scores+exp for
        # ktile i, AV for ktile i-1 (bank av01) and i-2 (bank av23) -- the
        # lag keeps the in-order PE from blocking on the just-issued exp.
        # Projection / out-projection filler work is cost-paced between
        # ktiles so the PE never starves while pacing behind ACT. -----
        pend_out = None    # chunk index awaiting emit_out (pair 3)
        carry = []         # (thunk, deadline-ktile) for window (3, 0)
        w_nxt = None
        for p in range(NPAIR):
            if p + 1 < NPAIR:
                w_nxt = load_w(p + 1)
                items = [(500, (lambda pp=p + 1, t=t: vproj_t(pp, t)))
                         for t in range(NKT)]
                items += [(1740, (lambda pp=p + 1, j=j, w=which,
                                  ws=w_nxt[which]:
                           qkproj_slice(pp, j, w, ws)))
                          for j in range(NCH) for which in range(2)]
                if p == 2:
                    # window (3,0) has no filler work of its own: carry
                    # pair 3's last qkproj slices there, each emitted
                    # before the ktile that first consumes its k-slice
                    carry = [(items[20][1], 4), (items[21][1], 6),
                             (items[22][1], 8), (items[23][1], 10)]
                    items = items[:20]
            else:
                items = []
            n_items = len(items)
            emitted = 0
            nwin = NCH if p > 0 else NCH - 1
            denom = max(1, nwin * NKT - 6)
            it_count = 0

            for c in range(NCH):
                # fillers: (cost_ns, thunk) of PE work to spread between
                # the score units
                fillers = []
                if pend_out is not None:
                    fillers += [(220, u) if k < NPAIR else (870, u)
                                for k, u in enumerate(emit_out_units(pend_out))]
                    pend_out = None
                if items and not (p == 0 and c == 0):
                    it_count += NKT
                    want = min(n_items, (it_count * n_items) // denom)
                    while emitted < want:
                        fillers.append(items[emitted])
                        emitted += 1

                pt_cur = ptp.tile([128, NKT, 2 * CH], bf16, tag="pt")
                av_t = [ps.tile([128, 2, 130], f32, tag="av", name=f"av{h}")
                        for h in range(2)]
                rcp_t = rcpp.tile([128, 2, 4], f32, tag="rcp")
                total = sum(cn for cn, _ in fillers)
                spent = 0
                for i in range(NKT):
                    if p == 0 and c == 0:
                        # inline projections for pair 0, aligned with the
                        # ktile order scores consume them in; the first
                        # slices run half-width so the leading matmuls only
                        # wait on the first half-slice x DMA
                        if i % 4 == 0:
                            hv = 2 if i == 0 else 1
                            qkproj_slice(0, i // 4, 0, w_cur[0], halves=hv)
                            qkproj_slice(0, i // 4, 1, w_cur[1], halves=hv)
                        vproj_t(0, i)
                    if p == NPAIR - 1 and c == 0:
                        while carry and carry[0][1] <= i:
                            carry.pop(0)[0]()
                    score_unit(p, c, i, pt_cur)
                    if i >= 2:
                        av_ktile(p, av_t, 0, i - 2, pt_cur,
                                 first=(i == 2), last=False)
                    if i >= 3:
                        av_ktile(p, av_t, 1, i - 3, pt_cur,
                                 first=(i == 3), last=False)
                    if i >= 1 and fillers:
                        target = (total * i) // (NKT - 1)
                        while fillers and spent < target:
                            cn, f = fillers.pop(0)
                            f()
                            spent += cn
                for k in (NKT - 2, NKT - 1):
                    av_ktile(p, av_t, 0, k, pt_cur, first=False,
                             last=(k == NKT - 1))
                for k in (NKT - 3, NKT - 2, NKT - 1):
                    av_ktile(p, av_t, 1, k, pt_cur, first=False,
                             last=(k == NKT - 1))
                for _, f in fillers:
                    f()
                norm_chunk(p, c, av_t, rcp_t)
                if p == NPAIR - 1 and c < NCH - 1:
                    pend_out = c

            while emitted < n_items:
                items[emitted][1]()
                emitted += 1
            w_cur = w_nxt

        # tail: out-projection for the last chunk
        for u in emit_out_units(NCH - 1):
            u()

        if _DEBUG:
            nc.sync.dma_start(out=dbg_qkT[:], in_=qkT[:])
            nc.sync.dma_start(out=dbg_v[:], in_=v_sb[:])
            nc.sync.dma_start(out=dbg_o[:], in_=o_sb[:])

    nc.compile()
    return nc


def _get_nc():
    if "nc" not in _CACHE:
        _CACHE["nc"] = _build()
    return _CACHE["nc"]


def _bf16(a):
    import ml_dtypes
    return np.ascontiguousarray(
        np.asarray(a, dtype=np.float32).astype(ml_dtypes.bfloat16))


def make_in_map(xT, wqT, wkT, wvT, woT, bq, bk, bv):
    """Pack one core's inputs into the kernel's tiled DRAM layouts.

    xT: [D, S]; wqT/wkT/wvT: [D, FH] (W sections transposed);
    woT: [FH, D] (out_w columns transposed); biases: [FH].
    """
    D, FH, ND, NPAIR, NH, FHA, NCH, CH = (
        _D, _FH, _ND, _NPAIR, _NH, _FHA, _NCH, _CH)
    # augment v with a per-head ones column: wv gets zero columns, bv gets
    # 1.0 entries -> the broadcast bias add installs the ones column, whose
    # AV accumulation yields the softmax denominators for free
    wva = np.zeros((D, FHA), dtype=np.float32)
    bva = np.zeros((1, FHA), dtype=np.float32)
    for h in range(NH):
        wva[:, h * 65:h * 65 + 64] = np.asarray(wvT)[:, h * 64:(h + 1) * 64]
        bva[0, h * 65:h * 65 + 64] = np.asarray(bv)[h * 64:(h + 1) * 64]
        bva[0, h * 65 + 64] = 1.0
    return {
        "xP": _bf16(np.asarray(xT).reshape(ND, 128, NCH, CH).transpose(2, 1, 0, 3)),
        "wq": _bf16(np.asarray(wqT).reshape(ND, 128, NPAIR, 128).transpose(2, 1, 0, 3)),
        "wk": _bf16(np.asarray(wkT).reshape(ND, 128, NPAIR, 128).transpose(2, 1, 0, 3)),
        "wv": _bf16(wva.reshape(ND, 128, FHA).transpose(1, 0, 2)),
        "wo": _bf16(np.asarray(woT).reshape(NPAIR, 128, D).transpose(1, 0, 2)),
        "bq": np.ascontiguousarray(
            np.asarray(bq, dtype=np.float32).reshape(NPAIR, 128).T),
        "bk": np.ascontiguousarray(
            np.asarray(bk, dtype=np.float32).reshape(NPAIR, 128).T),
        "bv": _bf16(bva),
        "idn": _bf16(np.eye(128)),
    }


def unpack_out(outp_tiled):
    """[NCH, ND, 128, CH] tiled partial -> [D, S] float32."""
    a = np.asarray(outp_tiled, dtype=np.float32)
    return a.transpose(1, 2, 0, 3).reshape(_D, _S)


def _shard_inputs(x, in_proj_weight, in_proj_bias, out_w):
    w = np.asarray(in_proj_weight)
    b = np.asarray(in_proj_bias)
    ow = np.asarray(out_w)
    in_maps = []
    for c in range(_NCORES):
        bi, g = divmod(c, 2)
        sl = slice(g * _FH, (g + 1) * _FH)
        in_maps.append(make_in_map(
            xT=np.asarray(x[bi]).T,
            wqT=w[0 * _D:1 * _D][sl].T,
            wkT=w[1 * _D:2 * _D][sl].T,
            wvT=w[2 * _D:3 * _D][sl].T,
            woT=ow[:, sl].T,
            bq=b[0 * _D:1 * _D][sl],
            bk=b[1 * _D:2 * _D][sl],
            bv=b[2 * _D:3 * _D][sl],
        ))
    return in_maps


LAST_RESULTS = None


def kernel(x, in_proj_weight, in_proj_bias, out_w, out_b):
    global LAST_RESULTS
    from concourse.bass_utils import run_bass_kernel_spmd
    import os

    nc = _get_nc()
    in_maps = _shard_inputs(x, in_proj_weight, in_proj_bias, out_w)
    trace = os.environ.get("BASS_TRACE", "0") not in ("", "0")
    res = run_bass_kernel_spmd(
        nc, in_maps, core_ids=list(range(_NCORES)), trace=trace
    )
    LAST_RESULTS = res
    out_b = np.asarray(out_b, dtype=np.float32)
    out = np.empty((_B, _S, _D), dtype=np.float32)
    for b in range(_B):
        part = (unpack_out(res.results[2 * b]["outp"])
                + unpack_out(res.results[2 * b + 1]["outp"]))
        out[b] = part.T + out_b
    return out
